# revision 1
# baseline (speedup 1.0000x reference)
"""KANLinear (no residual) Trainium2 kernel.

out[b,o] = sum_{i,g} B_g(x[b,i]) * W[o,i,g] where B_g are cubic B-spline
bases on a uniform grid (G=5, k=3, range [-1,1] -> 8 bases, knots
t_j = 0.4*j - 2.2).

Closed form used on-device: with u = 2.5*x + 5.5 - g and z = 2 - |u - 2|
(symmetry fold about the basis center),

    B_g(x) = relu(z*c1)^3 - relu((z-1)*c2)^3
    c1 = 6^(-1/3), c2 = (2/3)^(1/3)

which is exact for the cardinal cubic B-spline everywhere. Each hinge
cube is ONE custom 8-stage DVE op (mult, sub, neg, max, sub, relu, square,
mult) applied to an x^T tile; 16 such calls + one tensor_sub produce the 8
basis planes per 128-in-feature tile. The big matmul runs in float32r
(FP22 mantissa-truncated fp32) which streams at full PE rate for N>=256.

Sharding: data-parallel over tokens (4096 -> 512 per core on 8 cores),
spline_weight replicated; no collectives, host concatenates the shards.
"""

import numpy as np

N_CORES = 8
B_TOT = 4096
B_SHARD = B_TOT // N_CORES  # 512
IN_F = 1024
OUT_F = 1024
G = 8  # GRID_SIZE + SPLINE_ORDER
I_TILES = IN_F // 128  # 8
M_TILES = B_SHARD // 128  # 4
N_CHUNKS = OUT_F // 512  # 2

_C1 = float(6.0 ** (-1.0 / 3.0))
_C2 = float((2.0 / 3.0) ** (1.0 / 3.0))

_CACHE = {}


def _hinge_op():
    """Register (once) and return the custom DVE op

        out = cube(relu(imm2 - |in0*s0 - s1|))

    i.e. m = Src0*C0; t = m - C1; a = |t|; z = C2 - a; r = relu(z);
    out = r^3. Exactly 8 ALU stages on trn2 (abs is maxx(t, 0-t))."""
    if "op" in _CACHE:
        return _CACHE["op"]

    from concourse import dve_ops
    from concourse.dve_ops import DveOp
    from concourse.dve_spec import C0, C1, C2, Spec, Src0, Zero, lower, maxx, relu, sq
    from concourse.dve_uop import DveOpSpec

    name = "BSPLINE_HINGE_CUBE"

    def _ref(in0, in1, s0, s1, imm2):
        t = in0.astype(np.float32) * np.float32(s0) - np.float32(s1)
        z = (np.float32(imm2) - np.abs(t)).astype(np.float32)
        r = np.maximum(z, np.float32(0.0))
        return (r * r * r).astype(np.float32)

    m = Src0 * C0
    t = m - C1
    a = maxx(t, Zero - t)
    z = C2 - a
    r = relu(z)
    body = sq(r) * r
    spec = Spec(body=body, reference=_ref)

    if name not in dve_ops._SUB_OPCODE_FOR_NAME:
        row = dve_ops._CUSTOM_DVE_ROW_BASE + len(dve_ops.OPS)
        assert row < 0x20
        shas = {}
        for ver in ("v3", "v4"):
            try:
                tmp = DveOpSpec(
                    name=name, opcode=row, uops=lower(spec, ver=ver), rd1_en=False
                )
                shas[ver] = tmp.sha(ver)
            except Exception:
                pass
        op = DveOp(name, spec, subdim=False, uops_sha=shas)
        dve_ops.OPS.append(op)
        dve_ops._SUB_OPCODE_FOR_NAME[name] = row
        dve_ops.CUSTOM_DVE_SPECS[name] = spec
    else:
        op = next(o for o in dve_ops.OPS if o.name == name)

    _CACHE["op"] = op
    return op


def _build_nc():
    """Build the per-core Bass program (SPMD: identical on all 8 cores)."""
    if "nc" in _CACHE:
        return _CACHE["nc"]

    from concourse import bacc
    import concourse.mybir as mybir
    import concourse.tile as tile

    op = _hinge_op()

    f32 = mybir.dt.float32
    f32r = mybir.dt.float32r

    nc = bacc.Bacc(None, target_bir_lowering=False)

    x_t = nc.declare_dram_parameter("x_t", [IN_F, B_SHARD], f32, isOutput=False)
    w = nc.declare_dram_parameter("w", [G * IN_F, OUT_F], f32, isOutput=False)
    out = nc.declare_dram_parameter("out", [B_SHARD, OUT_F], f32, isOutput=True)

    with tile.TileContext(nc) as tc:
        with (
            tc.tile_pool(name="xp", bufs=3) as xp,
            tc.tile_pool(name="hp", bufs=2) as hp,
            tc.tile_pool(name="bp", bufs=2) as bp,
            tc.tile_pool(name="wp", bufs=4) as wp,
            tc.tile_pool(name="outp", bufs=2) as outp,
            tc.tile_pool(name="ps", bufs=1, space="PSUM") as ps,
        ):
            psum = [
                [
                    ps.tile([128, 512], f32, tag=f"ps_{m}_{n}", name=f"ps_{m}_{n}")
                    for n in range(N_CHUNKS)
                ]
                for m in range(M_TILES)
            ]

            for t in range(I_TILES):
                xt = xp.tile([128, B_SHARD], f32, tag="xt")
                nc.sync.dma_start(out=xt[:, :], in_=x_t[t * 128 : (t + 1) * 128, :])

                h1 = hp.tile([128, G * B_SHARD], f32, tag="h1")
                h2 = hp.tile([128, G * B_SHARD], f32, tag="h2")
                for s in range(G):
                    sl = slice(s * B_SHARD, (s + 1) * B_SHARD)
                    nc.vector._custom_dve(
                        op,
                        out=h1[:, sl],
                        in0=xt[:, :],
                        s0=2.5 * _C1,
                        s1=(s - 3.5) * _C1,
                        imm2=2.0 * _C1,
                    )
                    nc.vector._custom_dve(
                        op,
                        out=h2[:, sl],
                        in0=xt[:, :],
                        s0=2.5 * _C2,
                        s1=(s - 3.5) * _C2,
                        imm2=1.0 * _C2,
                    )
                bb = bp.tile([128, G * B_SHARD], f32, tag="bb")
                # write through an f32r view: FP32r matmult operands must come
                # from producers typed float32r (walrus birverifier)
                nc.vector.tensor_sub(bb[:, :].bitcast(f32r), h1[:, :], h2[:, :])

                for g in range(G):
                    wt = wp.tile([128, OUT_F], f32, tag="wt")
                    r0 = g * IN_F + t * 128
                    nc.sync.dma_start(
                        out=wt[:, :].bitcast(f32r), in_=w[r0 : r0 + 128, :].bitcast(f32r)
                    )
                    first = t == 0 and g == 0
                    last = t == I_TILES - 1 and g == G - 1
                    for m in range(M_TILES):
                        lhsT = bb[
                            :, g * B_SHARD + m * 128 : g * B_SHARD + (m + 1) * 128
                        ].bitcast(f32r)
                        for n in range(N_CHUNKS):
                            nc.tensor.matmul(
                                psum[m][n][:, :],
                                lhsT,
                                wt[:, n * 512 : (n + 1) * 512].bitcast(f32r),
                                start=first,
                                stop=last,
                            )

            for m in range(M_TILES):
                ot = outp.tile([128, OUT_F], f32, tag="ot")
                for n in range(N_CHUNKS):
                    nc.scalar.copy(
                        out=ot[:, n * 512 : (n + 1) * 512], in_=psum[m][n][:, :]
                    )
                nc.sync.dma_start(
                    out=out[m * 128 : (m + 1) * 128, :], in_=ot[:, :]
                )

    nc.finalize()
    _CACHE["nc"] = nc
    return nc


def _in_maps(x, w2):
    maps = []
    for c in range(N_CORES):
        xs = x[c * B_SHARD : (c + 1) * B_SHARD, :]
        maps.append({"x_t": np.ascontiguousarray(xs.T), "w": w2})
    return maps


def kernel(x, spline_weight, _trace=False):
    x = np.ascontiguousarray(np.asarray(x, dtype=np.float32))
    W = np.asarray(spline_weight, dtype=np.float32)
    assert x.shape == (B_TOT, IN_F) and W.shape == (OUT_F, IN_F, G)

    # w2[g*IN_F + i, o] = W[o, i, g]
    w2 = np.ascontiguousarray(W.transpose(2, 1, 0).reshape(G * IN_F, OUT_F))

    from concourse.bass_utils import run_bass_kernel_spmd

    nc = _build_nc()
    res = run_bass_kernel_spmd(nc, _in_maps(x, w2), list(range(N_CORES)), trace=_trace)
    out = np.concatenate(
        [np.asarray(res.results[c]["out"]) for c in range(N_CORES)], axis=0
    )
    if _trace:
        _CACHE["last_result"] = res
    return out.astype(np.float32, copy=False)



# revision 4
# speedup vs baseline: 1.1696x; 1.1696x over previous
"""KANLinear (no residual) Trainium2 kernel.

out[b,o] = sum_{i,g} B_g(x[b,i]) * W[o,i,g] where B_g are cubic B-spline
bases on a uniform grid (G=5, k=3, range [-1,1] -> 8 bases, knots
t_j = 0.4*j - 2.2).

Closed form used on-device: with u = 2.5*x + 5.5 - g and z = 2 - |u - 2|
(symmetry fold about the basis center),

    B_g(x) = relu(z*c1)^3 - relu((z-1)*c2)^3
    c1 = 6^(-1/3), c2 = (2/3)^(1/3)

which is exact for the cardinal cubic B-spline everywhere. Each hinge
cube is ONE custom 8-stage DVE op (mult, sub, neg, max, sub, relu, square,
mult) applied to an x^T tile; 16 such calls + one tensor_sub produce the 8
basis planes per 128-in-feature tile. The big matmul runs in float32r
(FP22 mantissa-truncated fp32) which streams at full PE rate for N>=256.

Sharding: data-parallel over tokens (4096 -> 512 per core on 8 cores),
spline_weight replicated; no collectives, host concatenates the shards.
"""

import numpy as np

N_CORES = 8
B_TOT = 4096
B_SHARD = B_TOT // N_CORES  # 512
IN_F = 1024
OUT_F = 1024
G = 8  # GRID_SIZE + SPLINE_ORDER
I_TILES = IN_F // 128  # 8
M_TILES = B_SHARD // 128  # 4
N_CHUNKS = OUT_F // 512  # 2

_C1 = float(6.0 ** (-1.0 / 3.0))
_C2 = float((2.0 / 3.0) ** (1.0 / 3.0))

_CACHE = {}


def _hinge_op():
    """Register (once) and return the custom DVE op

        out = cube(relu(imm2 - |in0*s0 - s1|))

    i.e. m = Src0*C0; t = m - C1; a = |t|; z = C2 - a; r = relu(z);
    out = r^3. Exactly 8 ALU stages on trn2 (abs is maxx(t, 0-t))."""
    if "op" in _CACHE:
        return _CACHE["op"]

    from concourse import dve_ops
    from concourse.dve_ops import DveOp
    from concourse.dve_spec import C0, C1, C2, Spec, Src0, Zero, lower, maxx, relu, sq
    from concourse.dve_uop import DveOpSpec

    name = "BSPLINE_HINGE_CUBE"

    def _ref(in0, in1, s0, s1, imm2):
        t = in0.astype(np.float32) * np.float32(s0) - np.float32(s1)
        z = (np.float32(imm2) - np.abs(t)).astype(np.float32)
        r = np.maximum(z, np.float32(0.0))
        return (r * r * r).astype(np.float32)

    m = Src0 * C0
    t = m - C1
    a = maxx(t, Zero - t)
    z = C2 - a
    r = relu(z)
    body = sq(r) * r
    spec = Spec(body=body, reference=_ref)

    if name not in dve_ops._SUB_OPCODE_FOR_NAME:
        row = dve_ops._CUSTOM_DVE_ROW_BASE + len(dve_ops.OPS)
        assert row < 0x20
        shas = {}
        for ver in ("v3", "v4"):
            try:
                tmp = DveOpSpec(
                    name=name, opcode=row, uops=lower(spec, ver=ver), rd1_en=False
                )
                shas[ver] = tmp.sha(ver)
            except Exception:
                pass
        op = DveOp(name, spec, subdim=False, uops_sha=shas)
        dve_ops.OPS.append(op)
        dve_ops._SUB_OPCODE_FOR_NAME[name] = row
        dve_ops.CUSTOM_DVE_SPECS[name] = spec
    else:
        op = next(o for o in dve_ops.OPS if o.name == name)

    _CACHE["op"] = op
    return op


def _build_nc():
    """Build the per-core Bass program (SPMD: identical on all 8 cores)."""
    if "nc" in _CACHE:
        return _CACHE["nc"]

    from concourse import bacc
    import concourse.mybir as mybir
    import concourse.tile as tile

    op = _hinge_op()

    f32 = mybir.dt.float32
    bf16 = mybir.dt.bfloat16

    nc = bacc.Bacc(None, target_bir_lowering=False)

    x_t = nc.declare_dram_parameter("x_t", [IN_F, B_SHARD], f32, isOutput=False)
    w = nc.declare_dram_parameter("w", [G * IN_F, OUT_F], bf16, isOutput=False)
    out = nc.declare_dram_parameter("out", [B_SHARD, OUT_F], f32, isOutput=True)

    with tile.TileContext(nc) as tc:
        with (
            tc.tile_pool(name="xp", bufs=3) as xp,
            tc.tile_pool(name="hp", bufs=2) as hp,
            tc.tile_pool(name="bp", bufs=2) as bp,
            tc.tile_pool(name="wp", bufs=4) as wp,
            tc.tile_pool(name="outp", bufs=2) as outp,
            tc.tile_pool(name="ps", bufs=1, space="PSUM") as ps,
        ):
            psum = [
                [
                    ps.tile([128, 512], f32, tag=f"ps_{m}_{n}", name=f"ps_{m}_{n}")
                    for n in range(N_CHUNKS)
                ]
                for m in range(M_TILES)
            ]

            for t in range(I_TILES):
                xt = xp.tile([128, B_SHARD], f32, tag="xt")
                nc.sync.dma_start(out=xt[:, :], in_=x_t[t * 128 : (t + 1) * 128, :])

                h1 = hp.tile([128, G * B_SHARD], f32, tag="h1")
                h2 = hp.tile([128, G * B_SHARD], f32, tag="h2")
                for s in range(G):
                    sl = slice(s * B_SHARD, (s + 1) * B_SHARD)
                    nc.vector._custom_dve(
                        op,
                        out=h1[:, sl],
                        in0=xt[:, :],
                        s0=2.5 * _C1,
                        s1=(s - 3.5) * _C1,
                        imm2=2.0 * _C1,
                    )
                    nc.vector._custom_dve(
                        op,
                        out=h2[:, sl],
                        in0=xt[:, :],
                        s0=2.5 * _C2,
                        s1=(s - 3.5) * _C2,
                        imm2=1.0 * _C2,
                    )
                bb = bp.tile([128, G * B_SHARD], bf16, tag="bb")
                nc.vector.tensor_sub(bb[:, :], h1[:, :], h2[:, :])

                for g in range(G):
                    wt = wp.tile([128, OUT_F], bf16, tag="wt")
                    r0 = g * IN_F + t * 128
                    nc.sync.dma_start(out=wt[:, :], in_=w[r0 : r0 + 128, :])
                    first = t == 0 and g == 0
                    last = t == I_TILES - 1 and g == G - 1
                    for m in range(M_TILES):
                        lhsT = bb[
                            :, g * B_SHARD + m * 128 : g * B_SHARD + (m + 1) * 128
                        ]
                        for n in range(N_CHUNKS):
                            nc.tensor.matmul(
                                psum[m][n][:, :],
                                lhsT,
                                wt[:, n * 512 : (n + 1) * 512],
                                start=first,
                                stop=last,
                            )

            for m in range(M_TILES):
                ot = outp.tile([128, OUT_F], f32, tag="ot")
                for n in range(N_CHUNKS):
                    nc.scalar.copy(
                        out=ot[:, n * 512 : (n + 1) * 512], in_=psum[m][n][:, :]
                    )
                nc.sync.dma_start(
                    out=out[m * 128 : (m + 1) * 128, :], in_=ot[:, :]
                )

    nc.finalize()
    _CACHE["nc"] = nc
    return nc


def _in_maps(x, w2):
    maps = []
    for c in range(N_CORES):
        xs = x[c * B_SHARD : (c + 1) * B_SHARD, :]
        maps.append({"x_t": np.ascontiguousarray(xs.T), "w": w2})
    return maps


def kernel(x, spline_weight, _trace=False):
    import ml_dtypes

    x = np.ascontiguousarray(np.asarray(x, dtype=np.float32))
    W = np.asarray(spline_weight, dtype=np.float32)
    assert x.shape == (B_TOT, IN_F) and W.shape == (OUT_F, IN_F, G)

    # w2[g*IN_F + i, o] = W[o, i, g]
    w2 = np.ascontiguousarray(
        W.transpose(2, 1, 0).reshape(G * IN_F, OUT_F).astype(ml_dtypes.bfloat16)
    )

    from concourse.bass_utils import run_bass_kernel_spmd

    nc = _build_nc()
    res = run_bass_kernel_spmd(nc, _in_maps(x, w2), list(range(N_CORES)), trace=_trace)
    out = np.concatenate(
        [np.asarray(res.results[c]["out"]) for c in range(N_CORES)], axis=0
    )
    if _trace:
        _CACHE["last_result"] = res
    return out.astype(np.float32, copy=False)



# revision 9
# speedup vs baseline: 1.2845x; 1.0982x over previous
"""KANLinear (no residual) Trainium2 kernel.

out[b,o] = sum_{i,g} B_g(x[b,i]) * W[o,i,g] where B_g are cubic B-spline
bases on a uniform grid (G=5, k=3, range [-1,1] -> 8 bases, knots
t_j = 0.4*j - 2.2).

Closed form used on-device: with u = 2.5*x + 5.5 - g and the fold
z = min(u, 4-u) (= 2 - |u-2|),

    B_g(x) = relu(z*c1)^3 - relu((z-1)*c2)^3
    c1 = 6^(-1/3),  c2 = (2/3)^(1/3)
    relu(z)   = relu(min(u, 4-u))      (min-of-two-affines, no abs needed)
    relu(z-1) = relu(min(u-1, 3-u))

which is exact for the cardinal cubic B-spline everywhere. Two custom DVE
ops per basis plane:
    HINGE1    (7 stages): h1 = cube(relu(min(x*s0 - s1, imm2 - x*s0)))
    HINGE2SUB (8 stages): B  = h1 - cube(relu(min(x*s0 - s1, imm2 - x*s0)))
The second op folds the h1-h2 subtraction, so no separate tensor_sub pass
is needed, and it writes the basis plane directly in bf16 for the matmul.

The big matmul runs in bf16 (1 PE cycle/row; fp32r on HW is a 2-pass mode
at ~2x the time). PSUM accumulates in fp32; output is evicted straight
from PSUM to DRAM by DMA. bf16 rounding of bases+weights gives ~2e-3
relative error vs the 2e-2 gate.

Sharding: data-parallel over tokens (4096 -> 512 per core on 8 cores),
spline_weight replicated (bf16, 16 MB/core streamed); no collectives,
host concatenates the shards.

Pipelining: per (i-tile t, basis g): 2 DVE ops (~1.35us) feed 8 matmuls
(~1.73us), so the DVE stays ahead of the PE and the tensor engine runs
back-to-back from ~2us after launch.
"""

import numpy as np

N_CORES = 8
B_TOT = 4096
B_SHARD = B_TOT // N_CORES  # 512
IN_F = 1024
OUT_F = 1024
G = 8  # GRID_SIZE + SPLINE_ORDER
I_TILES = IN_F // 128  # 8
M_TILES = B_SHARD // 128  # 4
N_CHUNKS = OUT_F // 512  # 2

_C1 = float(6.0 ** (-1.0 / 3.0))
_C2 = float((2.0 / 3.0) ** (1.0 / 3.0))

_CACHE = {}


def _dve_ops():
    """Register (once) and return the two custom DVE ops."""
    if "ops" in _CACHE:
        return _CACHE["ops"]

    from concourse import dve_ops
    from concourse.dve_ops import DveOp, has_src1
    from concourse.dve_spec import C0, C1, C2, Spec, Src0, Src1, lower, minn, relu, sq
    from concourse.dve_uop import DveOpSpec

    def cube(t):
        return sq(t) * t

    def _ref1(in0, in1, s0, s1, imm2):
        m = in0.astype(np.float32) * np.float32(s0)
        w = np.minimum(m - np.float32(s1), np.float32(imm2) - m)
        t = np.maximum(w, np.float32(0.0))
        return (t * t * t).astype(np.float32)

    def _ref2(in0, in1, s0, s1, imm2):
        m = in1.astype(np.float32) * np.float32(s0)
        w = np.minimum(m - np.float32(s1), np.float32(imm2) - m)
        t = np.maximum(w, np.float32(0.0))
        return (in0.astype(np.float32) - t * t * t).astype(np.float32)

    m1 = Src0 * C0
    spec1 = Spec(body=cube(relu(minn(m1 - C1, C2 - m1))), reference=_ref1)
    m2 = Src1 * C0
    spec2 = Spec(body=Src0 - cube(relu(minn(m2 - C1, C2 - m2))), reference=_ref2)

    ops = []
    for name, spec in (("BSPL_HINGE1", spec1), ("BSPL_HINGE2SUB", spec2)):
        if name not in dve_ops._SUB_OPCODE_FOR_NAME:
            row = dve_ops._CUSTOM_DVE_ROW_BASE + len(dve_ops.OPS)
            assert row < 0x20
            shas = {}
            for ver in ("v3", "v4"):
                try:
                    tmp = DveOpSpec(
                        name=name,
                        opcode=row,
                        uops=lower(spec, ver=ver),
                        rd1_en=has_src1(spec),
                    )
                    shas[ver] = tmp.sha(ver)
                except Exception:
                    pass
            op = DveOp(name, spec, subdim=False, uops_sha=shas)
            dve_ops.OPS.append(op)
            dve_ops._SUB_OPCODE_FOR_NAME[name] = row
            dve_ops.CUSTOM_DVE_SPECS[name] = spec
        else:
            op = next(o for o in dve_ops.OPS if o.name == name)
        ops.append(op)

    _CACHE["ops"] = tuple(ops)
    return _CACHE["ops"]


def _build_nc():
    """Build the per-core Bass program (SPMD: identical on all 8 cores)."""
    if "nc" in _CACHE:
        return _CACHE["nc"]

    from concourse import bacc
    import concourse.mybir as mybir
    import concourse.tile as tile

    op1, op2 = _dve_ops()

    f32 = mybir.dt.float32
    bf16 = mybir.dt.bfloat16

    nc = bacc.Bacc(None, target_bir_lowering=False)

    x_t = nc.declare_dram_parameter("x_t", [IN_F, B_SHARD], f32, isOutput=False)
    w = nc.declare_dram_parameter("w", [G * IN_F, OUT_F], bf16, isOutput=False)
    out = nc.declare_dram_parameter("out", [B_SHARD, OUT_F], f32, isOutput=True)

    with tile.TileContext(nc) as tc:
        with (
            tc.tile_pool(name="xp", bufs=2) as xp,
            tc.tile_pool(name="hp", bufs=3) as hp,
            tc.tile_pool(name="bp", bufs=4) as bp,
            tc.tile_pool(name="wp", bufs=4) as wp,
            tc.tile_pool(name="op", bufs=4) as op_,
            tc.tile_pool(name="ps", bufs=1, space="PSUM") as ps,
        ):
            psum = [
                [
                    ps.tile([128, 512], f32, tag=f"ps_{m}_{n}", name=f"ps_{m}_{n}")
                    for n in range(N_CHUNKS)
                ]
                for m in range(M_TILES)
            ]

            for t in range(I_TILES):
                xt = xp.tile([128, B_SHARD], f32, tag="xt")
                nc.sync.dma_start(out=xt[:, :], in_=x_t[t * 128 : (t + 1) * 128, :])

                for g in range(G):
                    h1 = hp.tile([128, B_SHARD], f32, tag="h1")
                    nc.vector._custom_dve(
                        op1,
                        out=h1[:, :],
                        in0=xt[:, :],
                        s0=2.5 * _C1,
                        s1=(g - 5.5) * _C1,
                        imm2=(g - 1.5) * _C1,
                    )
                    bb = bp.tile([128, B_SHARD], bf16, tag="bb")
                    nc.vector._custom_dve(
                        op2,
                        out=bb[:, :],
                        in0=h1[:, :],
                        in1=xt[:, :],
                        s0=2.5 * _C2,
                        s1=(g - 4.5) * _C2,
                        imm2=(g - 2.5) * _C2,
                    )

                    wt = wp.tile([128, OUT_F], bf16, tag="wt")
                    r0 = g * IN_F + t * 128
                    nc.sync.dma_start(out=wt[:, :], in_=w[r0 : r0 + 128, :])

                    first = t == 0 and g == 0
                    last = t == I_TILES - 1 and g == G - 1
                    for m in range(M_TILES):
                        lhsT = bb[:, m * 128 : (m + 1) * 128]
                        for n in range(N_CHUNKS):
                            nc.tensor.matmul(
                                psum[m][n][:, :],
                                lhsT,
                                wt[:, n * 512 : (n + 1) * 512],
                                start=first,
                                stop=last,
                            )

            for m in range(M_TILES):
                for n in range(N_CHUNKS):
                    ot = op_.tile([128, 512], f32, tag="ot")
                    if n == 0:
                        nc.scalar.copy(out=ot[:, :], in_=psum[m][n][:, :])
                    else:
                        nc.vector.tensor_copy(out=ot[:, :], in_=psum[m][n][:, :])
                    nc.sync.dma_start(
                        out=out[m * 128 : (m + 1) * 128, n * 512 : (n + 1) * 512],
                        in_=ot[:, :],
                    )

    nc.finalize()
    _CACHE["nc"] = nc
    return nc


def _in_maps(x, w2):
    maps = []
    for c in range(N_CORES):
        xs = x[c * B_SHARD : (c + 1) * B_SHARD, :]
        maps.append({"x_t": np.ascontiguousarray(xs.T), "w": w2})
    return maps


def kernel(x, spline_weight, _trace=False):
    import ml_dtypes

    x = np.ascontiguousarray(np.asarray(x, dtype=np.float32))
    W = np.asarray(spline_weight, dtype=np.float32)
    assert x.shape == (B_TOT, IN_F) and W.shape == (OUT_F, IN_F, G)

    # w2[g*IN_F + i, o] = W[o, i, g]
    w2 = np.ascontiguousarray(
        W.transpose(2, 1, 0).reshape(G * IN_F, OUT_F).astype(ml_dtypes.bfloat16)
    )

    from concourse.bass_utils import run_bass_kernel_spmd

    nc = _build_nc()
    res = run_bass_kernel_spmd(nc, _in_maps(x, w2), list(range(N_CORES)), trace=_trace)
    out = np.concatenate(
        [np.asarray(res.results[c]["out"]) for c in range(N_CORES)], axis=0
    )
    if _trace:
        _CACHE["last_result"] = res
    return out.astype(np.float32, copy=False)


# revision 11
# speedup vs baseline: 1.3039x; 1.0151x over previous
"""KANLinear (no residual) Trainium2 kernel.

out[b,o] = sum_{i,g} B_g(x[b,i]) * W[o,i,g] where B_g are cubic B-spline
bases on a uniform grid (G=5, k=3, range [-1,1] -> 8 bases, knots
t_j = 0.4*j - 2.2).

Closed form used on-device: with u = 2.5*x + 5.5 - g and the fold
z = min(u, 4-u) (= 2 - |u-2|),

    B_g(x) = relu(z*c1)^3 - relu((z-1)*c2)^3
    c1 = 6^(-1/3),  c2 = (2/3)^(1/3)
    relu(z)   = relu(min(u, 4-u))      (min-of-two-affines, no abs needed)
    relu(z-1) = relu(min(u-1, 3-u))

which is exact for the cardinal cubic B-spline everywhere. Two custom DVE
ops per basis plane:
    HINGE1    (7 stages): h1 = cube(relu(min(x*s0 - s1, imm2 - x*s0)))
    HINGE2SUB (8 stages): B  = h1 - cube(relu(min(x*s0 - s1, imm2 - x*s0)))
The second op folds the h1-h2 subtraction, so no separate tensor_sub pass
is needed, and it writes the basis plane directly in bf16 for the matmul.

The big matmul runs in bf16 (1 PE cycle/row; fp32r on HW is a 2-pass mode
at ~2x the time). PSUM accumulates in fp32; output is evicted straight
from PSUM to DRAM by DMA. bf16 rounding of bases+weights gives ~2e-3
relative error vs the 2e-2 gate.

Sharding: data-parallel over tokens (4096 -> 512 per core on 8 cores),
spline_weight replicated (bf16, 16 MB/core streamed); no collectives,
host concatenates the shards.

Pipelining: per (i-tile t, basis g): 2 DVE ops (~1.35us) feed 8 matmuls
(~1.73us), so the DVE stays ahead of the PE and the tensor engine runs
back-to-back from ~2us after launch.
"""

import numpy as np

N_CORES = 8
B_TOT = 4096
B_SHARD = B_TOT // N_CORES  # 512
IN_F = 1024
OUT_F = 1024
G = 8  # GRID_SIZE + SPLINE_ORDER
I_TILES = IN_F // 128  # 8
M_TILES = B_SHARD // 128  # 4
N_CHUNKS = OUT_F // 512  # 2

_C1 = float(6.0 ** (-1.0 / 3.0))
_C2 = float((2.0 / 3.0) ** (1.0 / 3.0))

_CACHE = {}


def _dve_ops():
    """Register (once) and return the two custom DVE ops."""
    if "ops" in _CACHE:
        return _CACHE["ops"]

    from concourse import dve_ops
    from concourse.dve_ops import DveOp, has_src1
    from concourse.dve_spec import C0, C1, C2, Spec, Src0, Src1, lower, minn, relu, sq
    from concourse.dve_uop import DveOpSpec

    def cube(t):
        return sq(t) * t

    def _ref1(in0, in1, s0, s1, imm2):
        m = in0.astype(np.float32) * np.float32(s0)
        w = np.minimum(m - np.float32(s1), np.float32(imm2) - m)
        t = np.maximum(w, np.float32(0.0))
        return (t * t * t).astype(np.float32)

    def _ref2(in0, in1, s0, s1, imm2):
        m = in1.astype(np.float32) * np.float32(s0)
        w = np.minimum(m - np.float32(s1), np.float32(imm2) - m)
        t = np.maximum(w, np.float32(0.0))
        return (in0.astype(np.float32) - t * t * t).astype(np.float32)

    m1 = Src0 * C0
    spec1 = Spec(body=cube(relu(minn(m1 - C1, C2 - m1))), reference=_ref1)
    m2 = Src1 * C0
    spec2 = Spec(body=Src0 - cube(relu(minn(m2 - C1, C2 - m2))), reference=_ref2)

    ops = []
    for name, spec in (("BSPL_HINGE1", spec1), ("BSPL_HINGE2SUB", spec2)):
        if name not in dve_ops._SUB_OPCODE_FOR_NAME:
            row = dve_ops._CUSTOM_DVE_ROW_BASE + len(dve_ops.OPS)
            assert row < 0x20
            shas = {}
            for ver in ("v3", "v4"):
                try:
                    tmp = DveOpSpec(
                        name=name,
                        opcode=row,
                        uops=lower(spec, ver=ver),
                        rd1_en=has_src1(spec),
                    )
                    shas[ver] = tmp.sha(ver)
                except Exception:
                    pass
            op = DveOp(name, spec, subdim=False, uops_sha=shas)
            dve_ops.OPS.append(op)
            dve_ops._SUB_OPCODE_FOR_NAME[name] = row
            dve_ops.CUSTOM_DVE_SPECS[name] = spec
        else:
            op = next(o for o in dve_ops.OPS if o.name == name)
        ops.append(op)

    _CACHE["ops"] = tuple(ops)
    return _CACHE["ops"]


def _build_nc():
    """Build the per-core Bass program (SPMD: identical on all 8 cores)."""
    if "nc" in _CACHE:
        return _CACHE["nc"]

    from concourse import bacc
    import concourse.mybir as mybir
    import concourse.tile as tile

    op1, op2 = _dve_ops()

    f32 = mybir.dt.float32
    bf16 = mybir.dt.bfloat16

    nc = bacc.Bacc(None, target_bir_lowering=False)

    x_t = nc.declare_dram_parameter("x_t", [IN_F, B_SHARD], f32, isOutput=False)
    w = nc.declare_dram_parameter("w", [G * IN_F, OUT_F], bf16, isOutput=False)
    out = nc.declare_dram_parameter("out", [B_SHARD, OUT_F], f32, isOutput=True)

    with tile.TileContext(nc) as tc:
        with (
            tc.tile_pool(name="xp", bufs=3) as xp,
            tc.tile_pool(name="hp", bufs=3) as hp,
            tc.tile_pool(name="bp", bufs=6) as bp,
            tc.tile_pool(name="wp", bufs=8) as wp,
            tc.tile_pool(name="op", bufs=8) as op_,
            tc.tile_pool(name="scr", bufs=1) as scrp,
            tc.tile_pool(name="ps", bufs=1, space="PSUM") as ps,
        ):
            psum = [
                [
                    ps.tile([128, 512], f32, tag=f"ps_{m}_{n}", name=f"ps_{m}_{n}")
                    for n in range(N_CHUNKS)
                ]
                for m in range(M_TILES)
            ]

            # PE warmup: ~3us of junk matmuls on a memset scratch tile so the
            # p-state governor reaches full clock before the real stream.
            scr = scrp.tile([128, 640], bf16, tag="scr")
            nc.gpsimd.memset(scr[:, :], 0.0)
            N_WARM = 14
            for i in range(N_WARM):
                nc.tensor.matmul(
                    psum[0][0][:, :],
                    scr[:, 0:128],
                    scr[:, 128:640],
                    start=i == 0,
                    stop=i == N_WARM - 1,
                )

            # wt DMAs alternate between the Sync and GpSimd DGE queues; xt
            # rides the Scalar queue, so neither steals weight bandwidth.
            for t in range(I_TILES):
                xt = xp.tile([128, B_SHARD], f32, tag="xt")
                nc.scalar.dma_start(out=xt[:, :], in_=x_t[t * 128 : (t + 1) * 128, :])

                for g in range(G):
                    wt = wp.tile([128, OUT_F], bf16, tag="wt")
                    r0 = g * IN_F + t * 128
                    weng = nc.sync if g % 2 == 0 else nc.gpsimd
                    weng.dma_start(out=wt[:, :], in_=w[r0 : r0 + 128, :])

                    first = t == 0 and g == 0
                    last = t == I_TILES - 1 and g == G - 1

                    # split the very first plane in half so the first matmuls
                    # start one DVE-op earlier
                    halves = (
                        ((0, 256), (256, 512)) if (t == 0 and g == 0) else ((0, 512),)
                    )
                    bbs = []
                    for lo, hi in halves:
                        h1 = hp.tile([128, hi - lo], f32, tag="h1")
                        nc.vector._custom_dve(
                            op1,
                            out=h1[:, :],
                            in0=xt[:, lo:hi],
                            s0=2.5 * _C1,
                            s1=(g - 5.5) * _C1,
                            imm2=(g - 1.5) * _C1,
                        )
                        bb = bp.tile([128, hi - lo], bf16, tag="bb")
                        nc.vector._custom_dve(
                            op2,
                            out=bb[:, :],
                            in0=h1[:, :],
                            in1=xt[:, lo:hi],
                            s0=2.5 * _C2,
                            s1=(g - 4.5) * _C2,
                            imm2=(g - 2.5) * _C2,
                        )
                        bbs.append((lo, hi, bb))

                    for m in range(M_TILES):
                        for lo, hi, bb in bbs:
                            if lo <= m * 128 < hi:
                                lhsT = bb[:, m * 128 - lo : (m + 1) * 128 - lo]
                        for n in range(N_CHUNKS):
                            nc.tensor.matmul(
                                psum[m][n][:, :],
                                lhsT,
                                wt[:, n * 512 : (n + 1) * 512],
                                start=first,
                                stop=last,
                            )

            for m in range(M_TILES):
                for n in range(N_CHUNKS):
                    ot = op_.tile([128, 512], f32, tag="ot")
                    if n == 0:
                        nc.scalar.copy(out=ot[:, :], in_=psum[m][n][:, :])
                    else:
                        nc.vector.tensor_copy(out=ot[:, :], in_=psum[m][n][:, :])
                    deng = nc.sync if (m * N_CHUNKS + n) % 2 == 0 else nc.gpsimd
                    deng.dma_start(
                        out=out[m * 128 : (m + 1) * 128, n * 512 : (n + 1) * 512],
                        in_=ot[:, :],
                    )

    nc.finalize()
    _CACHE["nc"] = nc
    return nc


def _in_maps(x, w2):
    maps = []
    for c in range(N_CORES):
        xs = x[c * B_SHARD : (c + 1) * B_SHARD, :]
        maps.append({"x_t": np.ascontiguousarray(xs.T), "w": w2})
    return maps


def kernel(x, spline_weight, _trace=False):
    import ml_dtypes

    x = np.ascontiguousarray(np.asarray(x, dtype=np.float32))
    W = np.asarray(spline_weight, dtype=np.float32)
    assert x.shape == (B_TOT, IN_F) and W.shape == (OUT_F, IN_F, G)

    # w2[g*IN_F + i, o] = W[o, i, g]
    w2 = np.ascontiguousarray(
        W.transpose(2, 1, 0).reshape(G * IN_F, OUT_F).astype(ml_dtypes.bfloat16)
    )

    from concourse.bass_utils import run_bass_kernel_spmd

    nc = _build_nc()
    res = run_bass_kernel_spmd(nc, _in_maps(x, w2), list(range(N_CORES)), trace=_trace)
    out = np.concatenate(
        [np.asarray(res.results[c]["out"]) for c in range(N_CORES)], axis=0
    )
    if _trace:
        _CACHE["last_result"] = res
    return out.astype(np.float32, copy=False)


# revision 18
# speedup vs baseline: 1.3133x; 1.0072x over previous
"""KANLinear (no residual) Trainium2 kernel.

out[b,o] = sum_{i,g} B_g(x[b,i]) * W[o,i,g] where B_g are cubic B-spline
bases on a uniform grid (G=5, k=3, range [-1,1] -> 8 bases, knots
t_j = 0.4*j - 2.2).

Closed form used on-device: with u = 2.5*x + 5.5 - g and the fold
z = min(u, 4-u) (= 2 - |u-2|),

    B_g(x) = relu(z*c1)^3 - relu((z-1)*c2)^3
    c1 = 6^(-1/3),  c2 = (2/3)^(1/3)
    relu(z)   = relu(min(u, 4-u))      (min-of-two-affines, no abs needed)
    relu(z-1) = relu(min(u-1, 3-u))

which is exact for the cardinal cubic B-spline everywhere. Two custom DVE
ops per basis plane:
    HINGE1    (7 stages): h1 = cube(relu(min(x*s0 - s1, imm2 - x*s0)))
    HINGE2SUB (8 stages): B  = h1 - cube(relu(min(x*s0 - s1, imm2 - x*s0)))
The second op folds the h1-h2 subtraction, so no separate tensor_sub pass
is needed, and it writes the basis plane directly in bf16 for the matmul.

The big matmul runs in bf16 (1 PE cycle/row; fp32r on HW is a 2-pass mode
at ~2x the time). PSUM accumulates in fp32; output is evicted straight
from PSUM to DRAM by DMA. bf16 rounding of bases+weights gives ~2e-3
relative error vs the 2e-2 gate.

Sharding: data-parallel over tokens (4096 -> 512 per core on 8 cores),
spline_weight replicated (bf16, 16 MB/core streamed); no collectives,
host concatenates the shards.

Pipelining: per (i-tile t, basis g): 2 DVE ops (~1.35us) feed 8 matmuls
(~1.73us), so the DVE stays ahead of the PE and the tensor engine runs
back-to-back from ~2us after launch.
"""

import numpy as np

N_CORES = 8
B_TOT = 4096
B_SHARD = B_TOT // N_CORES  # 512
IN_F = 1024
OUT_F = 1024
G = 8  # GRID_SIZE + SPLINE_ORDER
I_TILES = IN_F // 128  # 8
M_TILES = B_SHARD // 128  # 4
N_CHUNKS = OUT_F // 512  # 2

_C1 = float(6.0 ** (-1.0 / 3.0))
_C2 = float((2.0 / 3.0) ** (1.0 / 3.0))

_CACHE = {}


def _dve_ops():
    """Register (once) and return the two custom DVE ops."""
    if "ops" in _CACHE:
        return _CACHE["ops"]

    from concourse import dve_ops
    from concourse.dve_ops import DveOp, has_src1
    from concourse.dve_spec import C0, C1, C2, Spec, Src0, Src1, lower, minn, relu, sq
    from concourse.dve_uop import DveOpSpec

    def cube(t):
        return sq(t) * t

    def _ref1(in0, in1, s0, s1, imm2):
        m = in0.astype(np.float32) * np.float32(s0)
        w = np.minimum(m - np.float32(s1), np.float32(imm2) - m)
        t = np.maximum(w, np.float32(0.0))
        return (t * t * t).astype(np.float32)

    def _ref2(in0, in1, s0, s1, imm2):
        m = in1.astype(np.float32) * np.float32(s0)
        w = np.minimum(m - np.float32(s1), np.float32(imm2) - m)
        t = np.maximum(w, np.float32(0.0))
        return (in0.astype(np.float32) - t * t * t).astype(np.float32)

    m1 = Src0 * C0
    spec1 = Spec(body=cube(relu(minn(m1 - C1, C2 - m1))), reference=_ref1)
    m2 = Src1 * C0
    spec2 = Spec(body=Src0 - cube(relu(minn(m2 - C1, C2 - m2))), reference=_ref2)

    ops = []
    for name, spec in (("BSPL_HINGE1", spec1), ("BSPL_HINGE2SUB", spec2)):
        if name not in dve_ops._SUB_OPCODE_FOR_NAME:
            row = dve_ops._CUSTOM_DVE_ROW_BASE + len(dve_ops.OPS)
            assert row < 0x20
            shas = {}
            for ver in ("v3", "v4"):
                try:
                    tmp = DveOpSpec(
                        name=name,
                        opcode=row,
                        uops=lower(spec, ver=ver),
                        rd1_en=has_src1(spec),
                    )
                    shas[ver] = tmp.sha(ver)
                except Exception:
                    pass
            op = DveOp(name, spec, subdim=False, uops_sha=shas)
            dve_ops.OPS.append(op)
            dve_ops._SUB_OPCODE_FOR_NAME[name] = row
            dve_ops.CUSTOM_DVE_SPECS[name] = spec
        else:
            op = next(o for o in dve_ops.OPS if o.name == name)
        ops.append(op)

    _CACHE["ops"] = tuple(ops)
    return _CACHE["ops"]


def _build_nc():
    """Build the per-core Bass program (SPMD: identical on all 8 cores)."""
    if "nc" in _CACHE:
        return _CACHE["nc"]

    from concourse import bacc
    import concourse.mybir as mybir
    import concourse.tile as tile

    op1, op2 = _dve_ops()

    f32 = mybir.dt.float32
    bf16 = mybir.dt.bfloat16

    nc = bacc.Bacc(None, target_bir_lowering=False)

    x_t = nc.declare_dram_parameter("x_t", [IN_F, B_SHARD], f32, isOutput=False)
    w = nc.declare_dram_parameter("w", [G * IN_F, OUT_F], bf16, isOutput=False)
    out = nc.declare_dram_parameter("out", [B_SHARD, OUT_F], bf16, isOutput=True)

    with tile.TileContext(nc) as tc:
        with (
            tc.tile_pool(name="xp", bufs=3) as xp,
            tc.tile_pool(name="hp", bufs=3) as hp,
            tc.tile_pool(name="bp", bufs=6) as bp,
            tc.tile_pool(name="wp", bufs=8) as wp,
            tc.tile_pool(name="op", bufs=8) as op_,
            tc.tile_pool(name="scr", bufs=1) as scrp,
            tc.tile_pool(name="ps", bufs=1, space="PSUM") as ps,
        ):
            psum = [
                [
                    ps.tile([128, 512], f32, tag=f"ps_{m}_{n}", name=f"ps_{m}_{n}")
                    for n in range(N_CHUNKS)
                ]
                for m in range(M_TILES)
            ]

            # PE warmup: ~3us of junk matmuls on a memset scratch tile so the
            # p-state governor reaches full clock before the real stream.
            scr = scrp.tile([128, 640], bf16, tag="scr")
            nc.vector.memset(scr[:, :], 0.0)
            N_WARM = 14
            for i in range(N_WARM):
                nc.tensor.matmul(
                    psum[0][0][:, :],
                    scr[:, 0:128],
                    scr[:, 128:640],
                    start=i == 0,
                    stop=i == N_WARM - 1,
                )

            # wt DMAs alternate between the Sync and GpSimd DGE queues; xt
            # rides the Scalar queue (t=0 split across Vector+GpSimd so the
            # first bases start ~1.3us earlier), so neither steals weight
            # bandwidth.
            for t in range(I_TILES):
                xt = xp.tile([128, B_SHARD], f32, tag="xt")
                src = x_t[t * 128 : (t + 1) * 128, :]
                if t == 0:
                    nc.gpsimd.dma_start(out=xt[:, 0:256], in_=src[:, 0:256])
                    nc.scalar.dma_start(out=xt[:, 256:512], in_=src[:, 256:512])
                else:
                    nc.scalar.dma_start(out=xt[:, :], in_=src)

                for g in range(G):
                    wt = wp.tile([128, OUT_F], bf16, tag="wt")
                    r0 = g * IN_F + t * 128
                    weng = nc.sync if g % 2 == 0 else nc.gpsimd
                    weng.dma_start(out=wt[:, :], in_=w[r0 : r0 + 128, :])

                    first = t == 0 and g == 0
                    last = t == I_TILES - 1 and g == G - 1

                    # split the very first plane in half so the first matmuls
                    # start one DVE-op earlier
                    halves = (
                        ((0, 256), (256, 512)) if (t == 0 and g == 0) else ((0, 512),)
                    )
                    bbs = []
                    for lo, hi in halves:
                        h1 = hp.tile([128, hi - lo], f32, tag="h1")
                        nc.vector._custom_dve(
                            op1,
                            out=h1[:, :],
                            in0=xt[:, lo:hi],
                            s0=2.5 * _C1,
                            s1=(g - 5.5) * _C1,
                            imm2=(g - 1.5) * _C1,
                        )
                        bb = bp.tile([128, hi - lo], bf16, tag="bb")
                        nc.vector._custom_dve(
                            op2,
                            out=bb[:, :],
                            in0=h1[:, :],
                            in1=xt[:, lo:hi],
                            s0=2.5 * _C2,
                            s1=(g - 4.5) * _C2,
                            imm2=(g - 2.5) * _C2,
                        )
                        bbs.append((lo, hi, bb))

                    for m in range(M_TILES):
                        for lo, hi, bb in bbs:
                            if lo <= m * 128 < hi:
                                lhsT = bb[:, m * 128 - lo : (m + 1) * 128 - lo]
                        for n in range(N_CHUNKS):
                            nc.tensor.matmul(
                                psum[m][n][:, :],
                                lhsT,
                                wt[:, n * 512 : (n + 1) * 512],
                                start=first,
                                stop=last,
                            )

            dengs = [nc.sync, nc.gpsimd, nc.scalar]
            for m in range(M_TILES):
                for n in range(N_CHUNKS):
                    ot = op_.tile([128, 512], bf16, tag="ot")
                    if n == 0:
                        nc.scalar.copy(out=ot[:, :], in_=psum[m][n][:, :])
                    else:
                        nc.vector.tensor_copy(out=ot[:, :], in_=psum[m][n][:, :])
                    deng = dengs[(m * N_CHUNKS + n) % 3]
                    deng.dma_start(
                        out=out[m * 128 : (m + 1) * 128, n * 512 : (n + 1) * 512],
                        in_=ot[:, :],
                    )

    nc.finalize()
    _CACHE["nc"] = nc
    return nc


def _in_maps(x, w2):
    maps = []
    for c in range(N_CORES):
        xs = x[c * B_SHARD : (c + 1) * B_SHARD, :]
        maps.append({"x_t": np.ascontiguousarray(xs.T), "w": w2})
    return maps


def kernel(x, spline_weight, _trace=False):
    import ml_dtypes

    x = np.ascontiguousarray(np.asarray(x, dtype=np.float32))
    W = np.asarray(spline_weight, dtype=np.float32)
    assert x.shape == (B_TOT, IN_F) and W.shape == (OUT_F, IN_F, G)

    # w2[g*IN_F + i, o] = W[o, i, g]
    w2 = np.ascontiguousarray(
        W.transpose(2, 1, 0).reshape(G * IN_F, OUT_F).astype(ml_dtypes.bfloat16)
    )

    from concourse.bass_utils import run_bass_kernel_spmd

    nc = _build_nc()
    res = run_bass_kernel_spmd(nc, _in_maps(x, w2), list(range(N_CORES)), trace=_trace)
    out = np.concatenate(
        [np.asarray(res.results[c]["out"]) for c in range(N_CORES)], axis=0
    )
    if _trace:
        _CACHE["last_result"] = res
    return out.astype(np.float32)


# revision 19
# speedup vs baseline: 1.3144x; 1.0008x over previous
"""KANLinear (no residual) Trainium2 kernel.

out[b,o] = sum_{i,g} B_g(x[b,i]) * W[o,i,g] where B_g are cubic B-spline
bases on a uniform grid (G=5, k=3, range [-1,1] -> 8 bases, knots
t_j = 0.4*j - 2.2).

Closed form used on-device: with u = 2.5*x + 5.5 - g and the fold
z = min(u, 4-u) (= 2 - |u-2|),

    B_g(x) = relu(z*c1)^3 - relu((z-1)*c2)^3
    c1 = 6^(-1/3),  c2 = (2/3)^(1/3)
    relu(z)   = relu(min(u, 4-u))      (min-of-two-affines, no abs needed)
    relu(z-1) = relu(min(u-1, 3-u))

which is exact for the cardinal cubic B-spline everywhere. Two custom DVE
ops per basis plane:
    HINGE1    (7 stages): h1 = cube(relu(min(x*s0 - s1, imm2 - x*s0)))
    HINGE2SUB (8 stages): B  = h1 - cube(relu(min(x*s0 - s1, imm2 - x*s0)))
The second op folds the h1-h2 subtraction, so no separate tensor_sub pass
is needed, and it writes the basis plane directly in bf16 for the matmul.

The big matmul runs in bf16 (1 PE cycle/row; fp32r on HW is a 2-pass mode
at ~2x the time). PSUM accumulates in fp32; output is evicted straight
from PSUM to DRAM by DMA. bf16 rounding of bases+weights gives ~2e-3
relative error vs the 2e-2 gate.

Sharding: data-parallel over tokens (4096 -> 512 per core on 8 cores),
spline_weight replicated (bf16, 16 MB/core streamed); no collectives,
host concatenates the shards.

Pipelining: per (i-tile t, basis g): 2 DVE ops (~1.35us) feed 8 matmuls
(~1.73us), so the DVE stays ahead of the PE and the tensor engine runs
back-to-back from ~2us after launch.
"""

import numpy as np

N_CORES = 8
B_TOT = 4096
B_SHARD = B_TOT // N_CORES  # 512
IN_F = 1024
OUT_F = 1024
G = 8  # GRID_SIZE + SPLINE_ORDER
I_TILES = IN_F // 128  # 8
M_TILES = B_SHARD // 128  # 4
N_CHUNKS = OUT_F // 512  # 2

_C1 = float(6.0 ** (-1.0 / 3.0))
_C2 = float((2.0 / 3.0) ** (1.0 / 3.0))

_CACHE = {}


def _dve_ops():
    """Register (once) and return the two custom DVE ops."""
    if "ops" in _CACHE:
        return _CACHE["ops"]

    from concourse import dve_ops
    from concourse.dve_ops import DveOp, has_src1
    from concourse.dve_spec import C0, C1, C2, Spec, Src0, Src1, lower, minn, relu, sq
    from concourse.dve_uop import DveOpSpec

    def cube(t):
        return sq(t) * t

    def _ref1(in0, in1, s0, s1, imm2):
        m = in0.astype(np.float32) * np.float32(s0)
        w = np.minimum(m - np.float32(s1), np.float32(imm2) - m)
        t = np.maximum(w, np.float32(0.0))
        return (t * t * t).astype(np.float32)

    def _ref2(in0, in1, s0, s1, imm2):
        m = in1.astype(np.float32) * np.float32(s0)
        w = np.minimum(m - np.float32(s1), np.float32(imm2) - m)
        t = np.maximum(w, np.float32(0.0))
        return (in0.astype(np.float32) - t * t * t).astype(np.float32)

    m1 = Src0 * C0
    spec1 = Spec(body=cube(relu(minn(m1 - C1, C2 - m1))), reference=_ref1)
    m2 = Src1 * C0
    spec2 = Spec(body=Src0 - cube(relu(minn(m2 - C1, C2 - m2))), reference=_ref2)

    ops = []
    for name, spec in (("BSPL_HINGE1", spec1), ("BSPL_HINGE2SUB", spec2)):
        if name not in dve_ops._SUB_OPCODE_FOR_NAME:
            row = dve_ops._CUSTOM_DVE_ROW_BASE + len(dve_ops.OPS)
            assert row < 0x20
            shas = {}
            for ver in ("v3", "v4"):
                try:
                    tmp = DveOpSpec(
                        name=name,
                        opcode=row,
                        uops=lower(spec, ver=ver),
                        rd1_en=has_src1(spec),
                    )
                    shas[ver] = tmp.sha(ver)
                except Exception:
                    pass
            op = DveOp(name, spec, subdim=False, uops_sha=shas)
            dve_ops.OPS.append(op)
            dve_ops._SUB_OPCODE_FOR_NAME[name] = row
            dve_ops.CUSTOM_DVE_SPECS[name] = spec
        else:
            op = next(o for o in dve_ops.OPS if o.name == name)
        ops.append(op)

    _CACHE["ops"] = tuple(ops)
    return _CACHE["ops"]


def _build_nc():
    """Build the per-core Bass program (SPMD: identical on all 8 cores)."""
    if "nc" in _CACHE:
        return _CACHE["nc"]

    from concourse import bacc
    import concourse.mybir as mybir
    import concourse.tile as tile

    op1, op2 = _dve_ops()

    f32 = mybir.dt.float32
    bf16 = mybir.dt.bfloat16

    nc = bacc.Bacc(None, target_bir_lowering=False)

    x_t = nc.declare_dram_parameter("x_t", [IN_F, B_SHARD], f32, isOutput=False)
    w = nc.declare_dram_parameter("w", [G * IN_F, OUT_F], bf16, isOutput=False)
    out = nc.declare_dram_parameter("out", [B_SHARD, OUT_F], bf16, isOutput=True)

    with tile.TileContext(nc) as tc:
        with (
            tc.tile_pool(name="xp", bufs=3) as xp,
            tc.tile_pool(name="hp", bufs=4) as hp,
            tc.tile_pool(name="bp", bufs=8) as bp,
            tc.tile_pool(name="wp", bufs=8) as wp,
            tc.tile_pool(name="op", bufs=8) as op_,
            tc.tile_pool(name="scr", bufs=1) as scrp,
            tc.tile_pool(name="ps", bufs=1, space="PSUM") as ps,
        ):
            psum = [
                [
                    ps.tile([128, 512], f32, tag=f"ps_{m}_{n}", name=f"ps_{m}_{n}")
                    for n in range(N_CHUNKS)
                ]
                for m in range(M_TILES)
            ]

            # PE warmup: ~3us of junk matmuls on a memset scratch tile so the
            # p-state governor reaches full clock before the real stream.
            scr = scrp.tile([128, 640], bf16, tag="scr")
            nc.vector.memset(scr[:, :], 0.0)
            N_WARM = 10
            for i in range(N_WARM):
                nc.tensor.matmul(
                    psum[0][0][:, :],
                    scr[:, 0:128],
                    scr[:, 128:640],
                    start=i == 0,
                    stop=i == N_WARM - 1,
                )

            # wt DMAs alternate between the Sync and GpSimd DGE queues; xt
            # rides the Scalar queue (t=0 split across Vector+GpSimd so the
            # first bases start ~1.3us earlier), so neither steals weight
            # bandwidth.
            for t in range(I_TILES):
                xt = xp.tile([128, B_SHARD], f32, tag="xt")
                src = x_t[t * 128 : (t + 1) * 128, :]
                if t == 0:
                    nc.gpsimd.dma_start(out=xt[:, 0:256], in_=src[:, 0:256])
                    nc.sync.dma_start(out=xt[:, 256:512], in_=src[:, 256:512])
                else:
                    nc.scalar.dma_start(out=xt[:, :], in_=src)

                for g in range(G):
                    wt = wp.tile([128, OUT_F], bf16, tag="wt")
                    r0 = g * IN_F + t * 128
                    weng = nc.sync if g % 2 == 0 else nc.gpsimd
                    weng.dma_start(out=wt[:, :], in_=w[r0 : r0 + 128, :])

                    first = t == 0 and g == 0
                    last = t == I_TILES - 1 and g == G - 1

                    # split the very first plane in half so the first matmuls
                    # start one DVE-op earlier
                    halves = (
                        ((0, 256), (256, 512)) if (t == 0 and g == 0) else ((0, 512),)
                    )
                    bbs = []
                    for lo, hi in halves:
                        h1 = hp.tile([128, hi - lo], f32, tag="h1")
                        nc.vector._custom_dve(
                            op1,
                            out=h1[:, :],
                            in0=xt[:, lo:hi],
                            s0=2.5 * _C1,
                            s1=(g - 5.5) * _C1,
                            imm2=(g - 1.5) * _C1,
                        )
                        bb = bp.tile([128, hi - lo], bf16, tag="bb")
                        nc.vector._custom_dve(
                            op2,
                            out=bb[:, :],
                            in0=h1[:, :],
                            in1=xt[:, lo:hi],
                            s0=2.5 * _C2,
                            s1=(g - 4.5) * _C2,
                            imm2=(g - 2.5) * _C2,
                        )
                        bbs.append((lo, hi, bb))

                    for m in range(M_TILES):
                        for lo, hi, bb in bbs:
                            if lo <= m * 128 < hi:
                                lhsT = bb[:, m * 128 - lo : (m + 1) * 128 - lo]
                        for n in range(N_CHUNKS):
                            nc.tensor.matmul(
                                psum[m][n][:, :],
                                lhsT,
                                wt[:, n * 512 : (n + 1) * 512],
                                start=first,
                                stop=last,
                            )

            dengs = [nc.sync, nc.gpsimd]
            for m in range(M_TILES):
                for n in range(N_CHUNKS):
                    ot = op_.tile([128, 512], bf16, tag="ot")
                    if n == 0:
                        nc.scalar.copy(out=ot[:, :], in_=psum[m][n][:, :])
                    else:
                        nc.vector.tensor_copy(out=ot[:, :], in_=psum[m][n][:, :])
                    deng = dengs[(m * N_CHUNKS + n) % 2]
                    deng.dma_start(
                        out=out[m * 128 : (m + 1) * 128, n * 512 : (n + 1) * 512],
                        in_=ot[:, :],
                    )

    nc.finalize()
    _CACHE["nc"] = nc
    return nc


def _in_maps(x, w2):
    maps = []
    for c in range(N_CORES):
        xs = x[c * B_SHARD : (c + 1) * B_SHARD, :]
        maps.append({"x_t": np.ascontiguousarray(xs.T), "w": w2})
    return maps


def kernel(x, spline_weight, _trace=False):
    import ml_dtypes

    x = np.ascontiguousarray(np.asarray(x, dtype=np.float32))
    W = np.asarray(spline_weight, dtype=np.float32)
    assert x.shape == (B_TOT, IN_F) and W.shape == (OUT_F, IN_F, G)

    # w2[g*IN_F + i, o] = W[o, i, g]
    w2 = np.ascontiguousarray(
        W.transpose(2, 1, 0).reshape(G * IN_F, OUT_F).astype(ml_dtypes.bfloat16)
    )

    from concourse.bass_utils import run_bass_kernel_spmd

    nc = _build_nc()
    res = run_bass_kernel_spmd(nc, _in_maps(x, w2), list(range(N_CORES)), trace=_trace)
    out = np.concatenate(
        [np.asarray(res.results[c]["out"]) for c in range(N_CORES)], axis=0
    )
    if _trace:
        _CACHE["last_result"] = res
    return out.astype(np.float32)
